# revision 23
# baseline (speedup 1.0000x reference)
"""Self-contained TRN2 Bass kernel for the RGCN message-passing problem.

kernel(**inputs) takes the FULL unsharded inputs (text, src, dst, rel,
bases, comp, bias), shards destination rows across the 8 NeuronCores,
runs the SPMD Bass program via run_bass_kernel_spmd, and returns the
full [64, 512, 256] float32 output.

v8: uniform slot structure (272 slots x 16 dst rows x <=128 edges, one
gather chunk per slot) with host-side LPT bin-packing of dst rows into
slots -- cuts gather padding from ~25% to ~6% and makes the program
fully static (one compile, SPMD-identical across cores).  The one-hot
is half as wide (NW=48) and is streamed DURING the gather phase behind
gidx instead of blocking it.  Gather calls batch 2048 indices
(single_packet off) across the 4 SWDGE queues with 8 in-flight
buffers; the DMA descriptor ring carveout is doubled.
"""

import numpy as np
import ml_dtypes

import concourse.bass as bass
import concourse.tile as tile
from concourse import bacc, mybir

F = 256       # in features
O = 256       # out features
NB = 3        # bases
W = 16        # dst rows per slot
GRP = 8       # slots per stage-2 output group (GRP*W = 128 rows)
NSLOT_TRY = (264, 272, 288, 320, 512)  # slot counts to try (LPT pack)
NW = NB * W   # one-hot width per chunk (48)
DW = GRP * W  # dst rows per stage-2 group (128)
CPC = 8       # chunks per gather call (1024 idxs)
FIRST = 4     # chunks in each of the first two calls (fast ramp; they
              # drain concurrently on different queues)
GBUFS = 20    # gather-call tiles in flight
NQ = 4        # SWDGE queues (Q7 core pairs)
WARMUP_MM = 24  # dummy matmuls to keep the PE HAM clock warm at start


def plan_calls(nslot):
    calls = [(0, FIRST), (FIRST, FIRST)]
    lo = 2 * FIRST
    while lo < nslot:
        n = min(CPC, nslot - lo)
        calls.append((lo, n))
        lo += n
    return calls


def build_program(n_nodes, n_cores=8, bias_nonzero=False, nslot=264):
    NG = nslot // GRP
    calls = plan_calls(nslot)
    epad = nslot * 128

    bf16 = mybir.dt.bfloat16
    f32 = mybir.dt.float32
    i16 = mybir.dt.int16

    # bf16 DRAM I/O breaks NEFF load under the PJRT path; all bf16 payloads
    # travel as int16 containers and are bitcast on-chip.
    nc = bacc.Bacc("TRN2", target_bir_lowering=False, debug=False,
                   num_devices=n_cores, num_swdge_queues=NQ,
                   dynamic_dma_scratch_size=32768)
    h_d = nc.dram_tensor("h", [n_nodes, F], i16, kind="ExternalInput").ap()
    gidx_d = nc.dram_tensor("gidx", [128, epad // 16], i16,
                            kind="ExternalInput").ap()
    w1h_d = nc.dram_tensor("w1h", [128, nslot, NW], i16,
                           kind="ExternalInput").ap()
    bases_d = nc.dram_tensor("bases", [128, NB * 2 * O], i16,
                             kind="ExternalInput").ap()
    bias_d = nc.dram_tensor("bias", [1, O], i16, kind="ExternalInput").ap()
    out_d = nc.dram_tensor("out", [NG * DW, O], i16,
                           kind="ExternalOutput").ap()

    with tile.TileContext(nc) as tc:
        with (
            tc.tile_pool(name="const", bufs=1) as cpool,
            tc.tile_pool(name="gather", bufs=GBUFS) as gpool,
            tc.tile_pool(name="abt", bufs=3) as apool,
            tc.tile_pool(name="ost", bufs=4) as opool,
            tc.tile_pool(name="ps1", bufs=2, space="PSUM") as ps1,
            tc.tile_pool(name="ps2", bufs=2, space="PSUM") as ps2,
            tc.tile_pool(name="psw", bufs=1, space="PSUM") as psw,
        ):
            # ---- prologue ----
            # gidx is issued FIRST on the sync HWDGE ring so the gathers
            # (gated on it) start as early as possible; the one-hot slices
            # stream behind it, overlapping the gather phase (they only
            # gate stage-1 of their own chunks, not the gathers).
            gidx_sb = cpool.tile([128, epad // 16], i16)
            nc.sync.dma_start(gidx_sb[:], gidx_d[:])
            bases_i = cpool.tile([128, NB, 2, O], i16)
            nc.sync.dma_start(
                bases_i[:].rearrange("p a h o -> p (a h o)"), bases_d[:])
            bias_i = cpool.tile([1, O], i16)
            nc.sync.dma_start(bias_i[:], bias_d[:])
            wslices = []
            nsl = 2
            for s in range(nsl):
                lo = nslot * s // nsl
                hi = nslot * (s + 1) // nsl
                t = cpool.tile([128, hi - lo, NW], i16)
                nc.sync.dma_start(t[:], w1h_d[:, lo:hi, :])
                wslices.append((lo, hi, t))

            def w1h_ap(j):
                for lo, hi, t in wslices:
                    if lo <= j < hi:
                        return t[:, j - lo, :]
                raise AssertionError(j)

            bias_sb = bias_i[:].bitcast(bf16)
            ones_sb = cpool.tile([1, DW], bf16)
            nc.vector.memset(ones_sb[:], 1.0)

            # keep the PE HAM clock warm through the prologue
            wps = psw.tile([DW, O], f32)
            for _ in range(WARMUP_MM):
                nc.tensor.matmul(wps[:], ones_sb[:], bias_sb,
                                 start=True, stop=True)

            # ---- main pipeline ----
            nidx_regs = {}
            for glo, ncall in calls:
                n = ncall * 128
                if n not in nidx_regs:
                    nidx_regs[n] = nc.gpsimd.to_reg(n)
            def stage2(grp, abt):
                p2 = ps2.tile([DW, O], f32, tag="p2", name="p2")
                if bias_nonzero:
                    nc.tensor.matmul(p2[:], ones_sb[:], bias_sb,
                                     start=True, stop=False)
                for b in range(NB):
                    for h in range(2):
                        nc.tensor.matmul(
                            p2[:],
                            abt[h][:, b, :],
                            bases_i[:, b, h, :].bitcast(bf16),
                            start=(not bias_nonzero and b == 0 and h == 0),
                            stop=(b == NB - 1 and h == 1))
                osb = opool.tile([DW, O], bf16, tag="osb", name="osb")
                nc.scalar.activation(osb[:], p2[:],
                                     mybir.ActivationFunctionType.Relu)
                nc.sync.dma_start(out_d[grp * DW:(grp + 1) * DW, :],
                                  osb[:].bitcast(i16))

            pg = None
            pending = []   # stage-2 deferred one group so the PE FIFO
            # never stalls inline on the cross-engine abt copies
            for g, (glo, ncall) in enumerate(calls):
                nidx = ncall * 128
                G = gpool.tile([128, CPC, F], i16, tag="G", name="G")
                # pads carry index 0 (a real row; their one-hot rows are
                # zero), so every gathered lane is valid
                nc.gpsimd.dma_gather(
                    G[:, 0:ncall, :], h_d[:],
                    gidx_sb[:, glo * 8:(glo + ncall) * 8],
                    nidx, nidx_regs[nidx], F, queue_num=g % NQ,
                    single_packet=(nidx <= 1024))
                for c in range(ncall):
                    j = glo + c
                    q = j % GRP
                    if q == 0:
                        # per-group PSUM accumulator: [f, half, slot*(b,w)].
                        # Padded to 512 f32 per half so each half occupies
                        # exactly one PSUM bank -- a matmul output region
                        # must never straddle a bank boundary.
                        pg = ps1.tile([128, 2, 512], f32,
                                      tag="pg", name="pg")
                    for h in range(2):
                        nc.tensor.matmul(
                            pg[:, h, q * NW:(q + 1) * NW],
                            G[:, c, h * 128:(h + 1) * 128].bitcast(bf16),
                            w1h_ap(j).bitcast(bf16),
                            start=True, stop=True)
                    if q == GRP - 1:
                        grp = j // GRP
                        abt = [apool.tile([128, NB, DW], bf16,
                                          tag=f"abt{h}", name=f"abt{h}")
                               for h in range(2)]
                        for h in range(2):
                            # one batched PSUM->SBUF copy per (group, half):
                            # [f, (q a b)] -> [f, a, (q b)]
                            src_sl = pg[:, h, 0:GRP * NW].rearrange(
                                "p (q a b) -> p a q b", q=GRP, a=NB)
                            dst_sl = abt[h][:].rearrange(
                                "p a (q b) -> p a q b", q=GRP)
                            if h == 0:
                                nc.vector.tensor_copy(dst_sl, src_sl)
                            else:
                                nc.scalar.copy(dst_sl, src_sl)
                        pending.append((grp, abt))
                        if len(pending) > 2:
                            stage2(*pending.pop(0))
            for args in pending:
                stage2(*args)

    nc.compile()
    return nc


class PackError(Exception):
    pass


def host_prep(src, dst, rel, comp, n_nodes, n_cores, nslot):
    """Pack dst rows into uniform slots (LPT), build gather indices and
    the streamed one-hot, plus the output row map."""
    import heapq

    NG = nslot // GRP
    dcore = n_nodes // n_cores
    epad = nslot * 128
    w_edge = comp[rel].astype(ml_dtypes.bfloat16)        # [E, NB]
    core_of = dst // dcore

    gidx = np.zeros((n_cores, epad), np.int16)
    w1h = np.zeros((n_cores, nslot, 128, NW), ml_dtypes.bfloat16)
    rowmap = np.full((n_cores, NG * DW), -1, np.int64)

    for k in range(n_cores):
        em = np.where(core_of == k)[0]
        rloc = (dst[em] - k * dcore).astype(np.int64)
        cnt = np.bincount(rloc, minlength=dcore)
        # LPT: biggest rows first into the emptiest (by edges) slot that
        # still has row capacity
        order_rows = np.argsort(-cnt, kind="stable")
        heap = [(0, s) for s in range(nslot)]
        heapq.heapify(heap)
        slot_rows = [[] for _ in range(nslot)]
        for r in order_rows:
            c = int(cnt[r])
            if not heap:
                raise PackError(f"nslot={nslot}: out of row slots")
            e, s = heapq.heappop(heap)
            if e + c > 128:
                raise PackError(f"nslot={nslot}: {e}+{c} > 128")
            slot_rows[s].append(int(r))
            if len(slot_rows[s]) < W:
                heapq.heappush(heap, (e + c, s))
        slot_of_row = np.full(dcore, -1, np.int64)
        pos_of_row = np.full(dcore, -1, np.int64)
        for s in range(nslot):
            for p, r in enumerate(slot_rows[s]):
                slot_of_row[r] = s
                pos_of_row[r] = p
                rowmap[k, (s // GRP) * DW + (s % GRP) * W + p] = \
                    k * dcore + r
        es = slot_of_row[rloc]
        ep = pos_of_row[rloc]
        eo = np.argsort(es, kind="stable")
        sl_cnt = np.bincount(es, minlength=nslot)
        sl_start = np.concatenate([[0], np.cumsum(sl_cnt)])
        within = np.arange(len(eo)) - sl_start[es[eo]]
        gpos = es[eo] * 128 + within
        gidx[k, gpos] = src[em[eo]].astype(np.int16)
        cols = ep[eo]
        for b in range(NB):
            w1h[k, es[eo], within, b * W + cols] = w_edge[em[eo], b]

    # wrap gidx: idx i -> partition i%16, slot i//16; replicate to 128 parts
    gidx_t = gidx.reshape(n_cores, epad // 16, 16).transpose(0, 2, 1)
    gidx_t = np.tile(gidx_t, (1, 8, 1)).copy()
    w1h_t = w1h.transpose(0, 2, 1, 3).copy()   # [cores, 128, nslot, NW]
    return gidx_t, w1h_t, rowmap


def rgcn_kernel(text, src, dst, rel, bases, comp, bias, n_cores=8,
                run_fn=None, nc_cache={}):
    """Full-input kernel: shard, run on 8 cores, reassemble output."""
    Bt, St, INF = text.shape
    n_nodes = Bt * St
    h = text.reshape(n_nodes, INF)

    src = np.asarray(src).astype(np.int64)
    dst = np.asarray(dst).astype(np.int64)
    rel = np.asarray(rel).astype(np.int64)
    bases_np = np.asarray(bases, np.float32)
    comp_np = np.asarray(comp, np.float32)
    bias_np = np.asarray(bias, np.float32)

    for nslot in NSLOT_TRY:
        try:
            gidx_t, w1h_t, rowmap = host_prep(
                src, dst, rel, comp_np, n_nodes, n_cores, nslot)
            break
        except PackError:
            continue
    else:
        raise RuntimeError("slot packing failed at every NSLOT")
    bias_nonzero = bool(np.any(bias_np))
    key = (n_nodes, n_cores, bias_nonzero, nslot)
    if key not in nc_cache:
        nc_cache[key] = build_program(n_nodes, n_cores,
                                      bias_nonzero=bias_nonzero,
                                      nslot=nslot)
    nc = nc_cache[key]

    h_bf = np.asarray(h, np.float32).astype(ml_dtypes.bfloat16).view(np.int16)
    bases_bf = (bases_np.astype(ml_dtypes.bfloat16).view(np.int16)
                .reshape(NB, 2, 128, O).transpose(2, 0, 1, 3)
                .reshape(128, NB * 2 * O).copy())
    bias_bf = bias_np.reshape(1, O).astype(ml_dtypes.bfloat16).view(np.int16)

    in_maps = [
        dict(h=h_bf, gidx=gidx_t[k], w1h=w1h_t[k].view(np.int16),
             bases=bases_bf, bias=bias_bf)
        for k in range(n_cores)
    ]
    from concourse.bass_utils import run_bass_kernel_spmd
    if run_fn is None:
        res = run_bass_kernel_spmd(nc, in_maps, list(range(n_cores)))
        outs = [res.results[k]["out"] for k in range(n_cores)]
    else:
        outs = run_fn(nc, in_maps)

    out = np.zeros((n_nodes, O), np.float32)
    for k in range(n_cores):
        ok = outs[k].view(ml_dtypes.bfloat16).astype(np.float32)
        m = rowmap[k]
        sel = m >= 0
        out[m[sel]] = ok[sel]
    return out.reshape(Bt, St, O)


_NC_CACHE = {}


def kernel(text, src, dst, rel, bases, comp, bias):
    out = rgcn_kernel(
        np.asarray(text, np.float32),
        np.asarray(src), np.asarray(dst), np.asarray(rel),
        np.asarray(bases, np.float32), np.asarray(comp, np.float32),
        np.asarray(bias, np.float32),
        n_cores=8, nc_cache=_NC_CACHE)
    return np.ascontiguousarray(out, np.float32)


# revision 24
# speedup vs baseline: 1.1526x; 1.1526x over previous
"""Self-contained TRN2 Bass kernel for the RGCN message-passing problem.

kernel(**inputs) takes the FULL unsharded inputs (text, src, dst, rel,
bases, comp, bias), shards destination rows across the 8 NeuronCores,
runs the SPMD Bass program via run_bass_kernel_spmd, and returns the
full [64, 512, 256] float32 output.

v8: uniform slot structure (272 slots x 16 dst rows x <=128 edges, one
gather chunk per slot) with host-side LPT bin-packing of dst rows into
slots -- cuts gather padding from ~25% to ~6% and makes the program
fully static (one compile, SPMD-identical across cores).  The one-hot
is half as wide (NW=48) and is streamed DURING the gather phase behind
gidx instead of blocking it.  Gather calls batch 2048 indices
(single_packet off) across the 4 SWDGE queues with 8 in-flight
buffers; the DMA descriptor ring carveout is doubled.
"""

import numpy as np
import ml_dtypes

import concourse.bass as bass
import concourse.tile as tile
from concourse import bacc, mybir

F = 256       # in features
O = 256       # out features
NB = 3        # bases
W = 16        # dst rows per slot
GRP = 8       # slots per stage-2 output group (GRP*W = 128 rows)
NSLOT_TRY = (264, 272, 288, 320, 512)  # slot counts to try (LPT pack)
NW = NB * W   # one-hot width per chunk (48)
DW = GRP * W  # dst rows per stage-2 group (128)
CPC = 8       # chunks per gather call (1024 idxs)
FIRST = 4     # chunks in each of the first two calls (fast ramp; they
              # drain concurrently on different queues)
GBUFS = 16    # gather-call tiles in flight
NQ = 4        # SWDGE queues (Q7 core pairs)
WARMUP_MM = 24  # dummy matmuls to keep the PE HAM clock warm at start


def plan_calls(nslot):
    calls = [(0, FIRST), (FIRST, FIRST)]
    lo = 2 * FIRST
    while lo < nslot:
        n = min(CPC, nslot - lo)
        calls.append((lo, n))
        lo += n
    return calls


def build_program(n_nodes, n_cores=8, bias_nonzero=False, nslot=264):
    NG = nslot // GRP
    calls = plan_calls(nslot)
    epad = nslot * 128

    bf16 = mybir.dt.bfloat16
    f32 = mybir.dt.float32
    i16 = mybir.dt.int16

    # bf16 DRAM I/O breaks NEFF load under the PJRT path; all bf16 payloads
    # travel as int16 containers and are bitcast on-chip.
    nc = bacc.Bacc("TRN2", target_bir_lowering=False, debug=False,
                   num_devices=n_cores, num_swdge_queues=NQ,
                   dynamic_dma_scratch_size=32768)
    h_d = nc.dram_tensor("h", [n_nodes, F], i16, kind="ExternalInput").ap()
    gidx_d = nc.dram_tensor("gidx", [128, epad // 16], i16,
                            kind="ExternalInput").ap()
    w1h_d = nc.dram_tensor("w1h", [128, nslot, NW], i16,
                           kind="ExternalInput").ap()
    bases_d = nc.dram_tensor("bases", [128, NB * 2 * O], i16,
                             kind="ExternalInput").ap()
    bias_d = nc.dram_tensor("bias", [1, O], i16, kind="ExternalInput").ap()
    out_d = nc.dram_tensor("out", [NG * DW, O], i16,
                           kind="ExternalOutput").ap()

    with tile.TileContext(nc) as tc:
        with (
            tc.tile_pool(name="const", bufs=1) as cpool,
            tc.tile_pool(name="gather", bufs=GBUFS) as gpool,
            tc.tile_pool(name="abt", bufs=3) as apool,
            tc.tile_pool(name="ost", bufs=4) as opool,
            tc.tile_pool(name="ps1", bufs=2, space="PSUM") as ps1,
            tc.tile_pool(name="ps2", bufs=2, space="PSUM") as ps2,
            tc.tile_pool(name="psw", bufs=1, space="PSUM") as psw,
        ):
            # ---- prologue ----
            # gidx is issued FIRST on the sync HWDGE ring so the gathers
            # (gated on it) start as early as possible; the one-hot slices
            # stream behind it, overlapping the gather phase (they only
            # gate stage-1 of their own chunks, not the gathers).
            gidx_sb = cpool.tile([128, epad // 16], i16)
            nc.sync.dma_start(gidx_sb[:], gidx_d[:])
            bases_i = cpool.tile([128, NB, 2, O], i16)
            nc.sync.dma_start(
                bases_i[:].rearrange("p a h o -> p (a h o)"), bases_d[:])
            bias_i = cpool.tile([1, O], i16)
            nc.sync.dma_start(bias_i[:], bias_d[:])
            wslices = []
            nsl = 8
            for s in range(nsl):
                lo = nslot * s // nsl
                hi = nslot * (s + 1) // nsl
                t = cpool.tile([128, hi - lo, NW], i16)
                wslices.append([lo, hi, t, False])

            def w1h_load(s):
                lo, hi, t, loaded = wslices[s]
                if not loaded:
                    nc.sync.dma_start(t[:], w1h_d[:, lo:hi, :])
                    wslices[s][3] = True

            w1h_load(0)
            w1h_load(1)

            def w1h_ap(j):
                for lo, hi, t, _ in wslices:
                    if lo <= j < hi:
                        return t[:, j - lo, :]
                raise AssertionError(j)

            bias_sb = bias_i[:].bitcast(bf16)
            ones_sb = cpool.tile([1, DW], bf16)
            nc.vector.memset(ones_sb[:], 1.0)

            # keep the PE HAM clock warm through the prologue
            wps = psw.tile([DW, O], f32)
            for _ in range(WARMUP_MM):
                nc.tensor.matmul(wps[:], ones_sb[:], bias_sb,
                                 start=True, stop=True)

            # ---- main pipeline ----
            nidx_regs = {}
            for glo, ncall in calls:
                n = ncall * 128
                if n not in nidx_regs:
                    nidx_regs[n] = nc.gpsimd.to_reg(n)
            def stage2(grp, abt):
                p2 = ps2.tile([DW, O], f32, tag="p2", name="p2")
                if bias_nonzero:
                    nc.tensor.matmul(p2[:], ones_sb[:], bias_sb,
                                     start=True, stop=False)
                for b in range(NB):
                    for h in range(2):
                        nc.tensor.matmul(
                            p2[:],
                            abt[h][:, b, :],
                            bases_i[:, b, h, :].bitcast(bf16),
                            start=(not bias_nonzero and b == 0 and h == 0),
                            stop=(b == NB - 1 and h == 1))
                osb = opool.tile([DW, O], bf16, tag="osb", name="osb")
                nc.scalar.activation(osb[:], p2[:],
                                     mybir.ActivationFunctionType.Relu)
                nc.sync.dma_start(out_d[grp * DW:(grp + 1) * DW, :],
                                  osb[:].bitcast(i16))

            pg = None
            pending = []   # stage-2 deferred one group so the PE FIFO
            # never stalls inline on the cross-engine abt copies
            for g, (glo, ncall) in enumerate(calls):
                # prefetch the one-hot slice needed ~4 calls ahead
                ahead = min(glo + 4 * CPC, nslot - 1)
                w1h_load(min(ahead * nsl // nslot, nsl - 1))
                nidx = ncall * 128
                G = gpool.tile([128, CPC, F], i16, tag="G", name="G")
                # pads carry index 0 (a real row; their one-hot rows are
                # zero), so every gathered lane is valid
                nc.gpsimd.dma_gather(
                    G[:, 0:ncall, :], h_d[:],
                    gidx_sb[:, glo * 8:(glo + ncall) * 8],
                    nidx, nidx_regs[nidx], F, queue_num=g % NQ,
                    single_packet=(nidx <= 1024))
                for c in range(ncall):
                    j = glo + c
                    q = j % GRP
                    if q == 0:
                        # per-group PSUM accumulator: [f, half, slot*(b,w)].
                        # Padded to 512 f32 per half so each half occupies
                        # exactly one PSUM bank -- a matmul output region
                        # must never straddle a bank boundary.
                        pg = ps1.tile([128, 2, 512], f32,
                                      tag="pg", name="pg")
                    for h in range(2):
                        nc.tensor.matmul(
                            pg[:, h, q * NW:(q + 1) * NW],
                            G[:, c, h * 128:(h + 1) * 128].bitcast(bf16),
                            w1h_ap(j).bitcast(bf16),
                            start=True, stop=True)
                    if q == GRP - 1:
                        grp = j // GRP
                        abt = [apool.tile([128, NB, DW], bf16,
                                          tag=f"abt{h}", name=f"abt{h}")
                               for h in range(2)]
                        for h in range(2):
                            # one batched PSUM->SBUF copy per (group, half):
                            # [f, (q a b)] -> [f, a, (q b)]
                            src_sl = pg[:, h, 0:GRP * NW].rearrange(
                                "p (q a b) -> p a q b", q=GRP, a=NB)
                            dst_sl = abt[h][:].rearrange(
                                "p a (q b) -> p a q b", q=GRP)
                            if h == 0:
                                nc.vector.tensor_copy(dst_sl, src_sl)
                            else:
                                nc.scalar.copy(dst_sl, src_sl)
                        pending.append((grp, abt))
                        if len(pending) > 2:
                            stage2(*pending.pop(0))
            for args in pending:
                stage2(*args)

    nc.compile()
    return nc


class PackError(Exception):
    pass


def host_prep(src, dst, rel, comp, n_nodes, n_cores, nslot):
    """Pack dst rows into uniform slots (LPT), build gather indices and
    the streamed one-hot, plus the output row map."""
    import heapq

    NG = nslot // GRP
    dcore = n_nodes // n_cores
    epad = nslot * 128
    w_edge = comp[rel].astype(ml_dtypes.bfloat16)        # [E, NB]
    core_of = dst // dcore

    gidx = np.zeros((n_cores, epad), np.int16)
    w1h = np.zeros((n_cores, nslot, 128, NW), ml_dtypes.bfloat16)
    rowmap = np.full((n_cores, NG * DW), -1, np.int64)

    for k in range(n_cores):
        em = np.where(core_of == k)[0]
        rloc = (dst[em] - k * dcore).astype(np.int64)
        cnt = np.bincount(rloc, minlength=dcore)
        # LPT: biggest rows first into the emptiest (by edges) slot that
        # still has row capacity
        order_rows = np.argsort(-cnt, kind="stable")
        heap = [(0, s) for s in range(nslot)]
        heapq.heapify(heap)
        slot_rows = [[] for _ in range(nslot)]
        for r in order_rows:
            c = int(cnt[r])
            if not heap:
                raise PackError(f"nslot={nslot}: out of row slots")
            e, s = heapq.heappop(heap)
            if e + c > 128:
                raise PackError(f"nslot={nslot}: {e}+{c} > 128")
            slot_rows[s].append(int(r))
            if len(slot_rows[s]) < W:
                heapq.heappush(heap, (e + c, s))
        slot_of_row = np.full(dcore, -1, np.int64)
        pos_of_row = np.full(dcore, -1, np.int64)
        for s in range(nslot):
            for p, r in enumerate(slot_rows[s]):
                slot_of_row[r] = s
                pos_of_row[r] = p
                rowmap[k, (s // GRP) * DW + (s % GRP) * W + p] = \
                    k * dcore + r
        es = slot_of_row[rloc]
        ep = pos_of_row[rloc]
        eo = np.argsort(es, kind="stable")
        sl_cnt = np.bincount(es, minlength=nslot)
        sl_start = np.concatenate([[0], np.cumsum(sl_cnt)])
        within = np.arange(len(eo)) - sl_start[es[eo]]
        gpos = es[eo] * 128 + within
        gidx[k, gpos] = src[em[eo]].astype(np.int16)
        cols = ep[eo]
        for b in range(NB):
            w1h[k, es[eo], within, b * W + cols] = w_edge[em[eo], b]

    # wrap gidx: idx i -> partition i%16, slot i//16; replicate to 128 parts
    gidx_t = gidx.reshape(n_cores, epad // 16, 16).transpose(0, 2, 1)
    gidx_t = np.tile(gidx_t, (1, 8, 1)).copy()
    w1h_t = w1h.transpose(0, 2, 1, 3).copy()   # [cores, 128, nslot, NW]
    return gidx_t, w1h_t, rowmap


def rgcn_kernel(text, src, dst, rel, bases, comp, bias, n_cores=8,
                run_fn=None, nc_cache={}):
    """Full-input kernel: shard, run on 8 cores, reassemble output."""
    Bt, St, INF = text.shape
    n_nodes = Bt * St
    h = text.reshape(n_nodes, INF)

    src = np.asarray(src).astype(np.int64)
    dst = np.asarray(dst).astype(np.int64)
    rel = np.asarray(rel).astype(np.int64)
    bases_np = np.asarray(bases, np.float32)
    comp_np = np.asarray(comp, np.float32)
    bias_np = np.asarray(bias, np.float32)

    for nslot in NSLOT_TRY:
        try:
            gidx_t, w1h_t, rowmap = host_prep(
                src, dst, rel, comp_np, n_nodes, n_cores, nslot)
            break
        except PackError:
            continue
    else:
        raise RuntimeError("slot packing failed at every NSLOT")
    bias_nonzero = bool(np.any(bias_np))
    key = (n_nodes, n_cores, bias_nonzero, nslot)
    if key not in nc_cache:
        nc_cache[key] = build_program(n_nodes, n_cores,
                                      bias_nonzero=bias_nonzero,
                                      nslot=nslot)
    nc = nc_cache[key]

    h_bf = np.asarray(h, np.float32).astype(ml_dtypes.bfloat16).view(np.int16)
    bases_bf = (bases_np.astype(ml_dtypes.bfloat16).view(np.int16)
                .reshape(NB, 2, 128, O).transpose(2, 0, 1, 3)
                .reshape(128, NB * 2 * O).copy())
    bias_bf = bias_np.reshape(1, O).astype(ml_dtypes.bfloat16).view(np.int16)

    in_maps = [
        dict(h=h_bf, gidx=gidx_t[k], w1h=w1h_t[k].view(np.int16),
             bases=bases_bf, bias=bias_bf)
        for k in range(n_cores)
    ]
    from concourse.bass_utils import run_bass_kernel_spmd
    if run_fn is None:
        res = run_bass_kernel_spmd(nc, in_maps, list(range(n_cores)))
        outs = [res.results[k]["out"] for k in range(n_cores)]
    else:
        outs = run_fn(nc, in_maps)

    out = np.zeros((n_nodes, O), np.float32)
    for k in range(n_cores):
        ok = outs[k].view(ml_dtypes.bfloat16).astype(np.float32)
        m = rowmap[k]
        sel = m >= 0
        out[m[sel]] = ok[sel]
    return out.reshape(Bt, St, O)


_NC_CACHE = {}


def kernel(text, src, dst, rel, bases, comp, bias):
    out = rgcn_kernel(
        np.asarray(text, np.float32),
        np.asarray(src), np.asarray(dst), np.asarray(rel),
        np.asarray(bases, np.float32), np.asarray(comp, np.float32),
        np.asarray(bias, np.float32),
        n_cores=8, nc_cache=_NC_CACHE)
    return np.ascontiguousarray(out, np.float32)
